# revision 17
# baseline (speedup 1.0000x reference)
"""Gaussian upsampling embedding kernel for Trainium2 (8 NeuronCores).

Data-parallel over the batch dim: 32 batches -> 4 slots per core, with
batches assigned to (core, slot) by sorted total-duration so each slot's
cross-core unions (spans, chunk count) stay tight.

Math (per batch b):
  c_i   = cumsum(durs)_i - durs_i/2          (gaussian centers)
  sig_i = durs_i/2 + 1e-6
  w[t,i] = 1/(sig_i*sqrt(2pi)) * exp(-((t+0.5-c_i)/sig_i)^2/2)
  out[t,:] = sum_i w[t,i]*embed[text_i] / sum_i w[t,i]          (t < total_dur)
  out[t,:] = embed[0]                                           (t >= total_dur)

Device pipeline per slot (engines overlap under Tile):
  ACT : w[i,t] = Derivative_Erf(s_i*tval[t_local] + b'_i) in bf16, over the
        span of t-chunks the char half contributes to (span offset folded
        into b' on host so one short iota serves all spans)
  PE  : O[t,:] = sum over char halves q of w_q[:,tchunk]^T @ Eg_q   (bf16)
        Eg_q[i,:] = amp_i * embed[text_i] + an amp column -> O[:,384] = S
  DVE : recip[t] = 1/S (two 128-row chunks per op via strided PSUM AP)
  DVE/ACT : out = O[:,:384]*recip -> bf16 (psum->sbuf copy fused with
        normalize; work split between the engines by a host-balanced
        schedule — DVE handles whole psum pairs in one tensor_tensor with
        a stride-0 broadcast recip AP, ACT handles single chunks via
        activation Copy+scale); output DMA flushed per psum pair from the
        otherwise-idle GpSimd queue to overlap the store. A dummy
        activation at t=0 preloads the Derivative_Erf table off the
        critical path.

Time-padding rows (t >= total_dur) and the f32 upcast are handled on the
host: those rows are exactly embed[0], so the device never computes them
(rows past the slot's max duration are skipped entirely; rows in computed
chunks may hold NaN from 0 * 1/0 and are overwritten).
"""

import os
import numpy as np
from contextlib import ExitStack

import ml_dtypes

_B, _T, _V, _D = 32, 256, 100, 384
_NC = 8
_BPC = _B // _NC    # batch slots per core
_EPS = np.float32(1e-6)
_MARGIN = 6.0       # |z| beyond which w is dropped (w < 1.6e-8: negligible)
_BF16 = ml_dtypes.bfloat16

# Set by kernel() after each run (for the local test harness).
LAST_RESULT = None


def _build_program(NTs, spans, maxspan, sched):
    """NTs[b] = number of 128-row t-chunks computed for slot b.
    spans[b][q] = (c_lo, c_hi) chunk range half q contributes to (union
    across cores). sched[b][g] in {'dve','act','split'} = normalize
    engine(s) for psum pair g."""
    import concourse.bass as bass
    import concourse.tile as tile
    from concourse import bacc, mybir

    f32 = mybir.dt.float32
    bf16 = mybir.dt.bfloat16
    AF = mybir.ActivationFunctionType
    _af_gauss = (
        AF.Exp if os.environ.get("GK_SIM_AF") else AF.Derivative_Erf
    )

    NTP = max(NTs) * 128

    nc = bacc.Bacc(
        "TRN2",
        target_bir_lowering=False,
        debug=False,
        num_devices=_NC,
    )

    coef = nc.dram_tensor("coef", [128, _BPC * 2 * 2], f32, kind="ExternalInput").ap()
    tval = nc.dram_tensor("tval", [128, maxspan], f32, kind="ExternalInput").ap()
    egp = nc.dram_tensor(
        "egp", [_BPC, 2, 128, _D + 2], bf16, kind="ExternalInput"
    ).ap()
    out = nc.dram_tensor("out", [_BPC, NTP, _D], bf16, kind="ExternalOutput").ap()

    with tile.TileContext(nc) as tc, ExitStack() as ctx:
        const = ctx.enter_context(tc.tile_pool(name="const", bufs=1))
        wpool = ctx.enter_context(tc.tile_pool(name="wT", bufs=8))
        opool = ctx.enter_context(tc.tile_pool(name="osb", bufs=6))
        rpool = ctx.enter_context(tc.tile_pool(name="recip", bufs=10))
        pso = ctx.enter_context(tc.tile_pool(name="pso", bufs=2, space="PSUM"))

        # tval/coef/first eg block arrive via the Activation queue (its
        # preamble drains earliest; DMA issue is only legal from gpsimd/
        # SP/Activation); tval is a host-shipped arange (the +0.5 frame
        # offset and span start are folded into the bias coefficients)
        tval_sb = const.tile([128, maxspan], f32)
        nc.sync.dma_start(tval_sb[:], tval[:])
        coef_sb = const.tile([128, _BPC * 2 * 2], f32)
        nc.sync.dma_start(coef_sb[:], coef[:])
        eg_sb = const.tile([128, _BPC * 2 * (_D + 2)], bf16)
        for bb in range(_BPC):
            w0 = bb * 2 * (_D + 2)
            eng = nc.sync
            eng.dma_start(
                eg_sb[:, w0 : w0 + 2 * (_D + 2)].rearrange(
                    "p (q d) -> p q d", q=2
                ),
                egp[bb].rearrange("q p d -> p q d"),
            )

        # preload the Derivative_Erf table with a dummy activation so the
        # 1.3us ACT_TABLE_LOAD overlaps the input DMAs (bias from a memset
        # tile: a float bias would pull in a DMA-backed const AP and delay
        # the table load behind the const DMA)
        tiny = const.tile([1, 6], f32)
        nc.vector.memset(tiny[:, 0:4], 0)
        nc.scalar.activation(
            tiny[:, 4:6], tiny[:, 0:2], _af_gauss,
            scale=1.0, bias=tiny[:, 2:3],
        )

        def cf(b, q, c):
            j = (b * 2 + q) * 2 + c
            return coef_sb[:, j : j + 1]

        def eg(b, q):
            j = (b * 2 + q) * (_D + 2)
            return eg_sb[:, j : j + _D + 2]

        for b in range(_BPC):
            NT = NTs[b]
            # Gaussian eval over each half's contributing span (bf16)
            wT = []
            for q in range(2):
                lo, hi = spans[b][q]
                n = (hi - lo) * 128
                w = wpool.tile([128, n], bf16, tag="wT")
                nc.scalar.activation(
                    w[:], tval_sb[:, :n], _af_gauss,
                    scale=cf(b, q, 0), bias=cf(b, q, 1),
                )
                wT.append(w)

            for g in range((NT + 3) // 4):
                ilist = [i for i in range(4 * g, 4 * g + 4) if i < NT]
                ng = len(ilist)
                po = pso.tile([128, 2048], f32, tag="pso")
                for j, i in enumerate(ilist):
                    dst = po[:, j * 512 : j * 512 + _D + 2]
                    qs = [
                        q
                        for q in range(2)
                        if spans[b][q][0] <= i < spans[b][q][1]
                    ]
                    assert qs, f"t-chunk {i} of slot {b} has no contribution"
                    for k, q in enumerate(qs):
                        o = (i - spans[b][q][0]) * 128
                        nc.tensor.matmul(
                            dst,
                            wT[q][:, o : o + 128],
                            eg(b, q),
                            start=(k == 0),
                            stop=(k == len(qs) - 1),
                        )
                rc = rpool.tile([128, 4], f32, tag="recip")
                nc.vector.reciprocal(
                    rc[:, :ng], po[:, _D : _D + 512 * (ng - 1) + 1 : 512]
                )
                ot = opool.tile([128, ng * _D], bf16, tag="osb")
                m = sched[b][g]            # ACT takes chunks [0, m)
                for j in range(m):
                    nc.scalar.activation(
                        ot[:, j * _D : (j + 1) * _D],
                        po[:, j * 512 : j * 512 + _D],
                        AF.Copy,
                        scale=rc[:, j : j + 1],
                    )
                k = ng - m                 # DVE takes chunks [m, ng)
                if k == 1:
                    nc.vector.tensor_scalar_mul(
                        ot[:, m * _D : (m + 1) * _D],
                        po[:, m * 512 : m * 512 + _D],
                        rc[:, m : m + 1],
                    )
                elif k >= 2:
                    nc.vector.tensor_tensor(
                        ot[:, m * _D :].rearrange("p (j d) -> p j d", d=_D),
                        po[:, m * 512 : m * 512 + k * 512].rearrange(
                            "p (j d) -> p j d", j=k
                        )[:, :, 0:_D],
                        rc[:, m : m + k].unsqueeze(2).broadcast_to(
                            [128, k, _D]
                        ),
                        mybir.AluOpType.mult,
                    )
                nc.gpsimd.dma_start(
                    out[b, 4 * g * 128 : (4 * g + ng) * 128].rearrange(
                        "(i p) d -> p i d", p=128
                    ),
                    ot[:].rearrange("p (i d) -> p i d", d=_D),
                )

    nc.compile()
    return nc


def _host_prep(text, durs, embed, Tt):
    """Sorted slot assignment, per-core input maps, spans, schedule."""
    text_i = np.asarray(text).astype(np.int64)          # [32, 256]
    durs_f = np.asarray(durs).astype(np.float32)        # [32, 256]
    embed = np.asarray(embed, dtype=np.float32)         # [100, 384]

    td = np.asarray(durs).astype(np.int64).sum(axis=-1)  # [32]
    order = np.argsort(td, kind="stable")                # slot-major ranks
    # batch at (core c, slot b) = order[b*8 + c]
    NTs = []
    for b in range(_BPC):
        mx = int(td[order[b * _NC : (b + 1) * _NC]].max())
        NTs.append(-(-mx // 128))

    csum = np.cumsum(durs_f, axis=-1, dtype=np.float32)
    c = csum - durs_f / 2.0                             # centers
    sig = durs_f / 2.0 + _EPS
    sq2 = np.float32(np.sqrt(2.0))
    s_coef = (1.0 / (sig * sq2)).astype(np.float32)
    b_coef = ((0.5 - c) / (sig * sq2)).astype(np.float32)
    amp = (1.0 / (2.0 * sq2 * sig)).astype(np.float32)

    # contribution spans per (slot, char-half) on the 128-chunk grid,
    # unioned across the 8 cores (SPMD-shared program)
    lo_t = (c - _MARGIN * sig).reshape(_B, 2, 128).min(axis=2)
    hi_t = (c + _MARGIN * sig + 1).reshape(_B, 2, 128).max(axis=2)
    spans = []
    for b in range(_BPC):
        ids = order[b * _NC : (b + 1) * _NC]
        NT = NTs[b]
        row = []
        for q in range(2):
            lo = max(0.0, float(lo_t[ids, q].min()))
            hi = min(float(NT * 128), float(hi_t[ids, q].max()))
            c_lo = max(0, min(int(lo) // 128, NT - 1))
            c_hi = max(c_lo + 1, min(-(-int(hi) // 128), NT))
            row.append((c_lo, c_hi))
        # coverage check: every chunk must get at least one matmul
        for i in range(NT):
            assert any(r[0] <= i < r[1] for r in row), (b, i, row)
        spans.append(tuple(row))
    spans = tuple(spans)
    maxspan = max((hi - lo) * 128 for row in spans for (lo, hi) in row)

    # normalize engine schedule per psum quad: sched[b][g] = m = number of
    # chunks ACT takes (from the front); DVE fuses the rest in one
    # tensor_tensor. Greedy balance of estimated busy ns (measured: ACT
    # chunk ~755, DVE fused ~400/chunk + 250, single ~613, recip ~165).
    # ACT also evaluates the gaussians.
    act_t = sum(
        (hi - lo) * 128 * 0.833 + 400.0 for row in spans for (lo, hi) in row
    )
    dve_t = 0.0
    sched = []
    for b in range(_BPC):
        row = []
        for g in range(-(-NTs[b] // 4)):
            ng = min(4, NTs[b] - 4 * g)
            best, cost = None, None
            for m in range(ng + 1):
                k = ng - m
                a = m * 755.0
                v = 165.0 + (0.0 if k == 0 else (613.0 if k == 1 else 400.0 * k + 250.0))
                c = max(act_t + a, dve_t + v)
                if cost is None or c < cost:
                    best, cost, ba, bv = m, c, a, v
            row.append(best)
            act_t += ba
            dve_t += bv
        sched.append(row)

    # coef layout: [128 partitions, (b, q, c)] with c = (s, b'),
    # b' = b + s * span_start so the short local iota can be used
    stack = np.stack([s_coef, b_coef], axis=-1)          # [32, 256, 2]
    stack = stack.reshape(_B, 2, 128, 2)                 # [32, q, p, c]

    # gathered, amplitude-folded embeddings + amp column (row-sum), bf16
    egp = np.zeros((_B, 2, 128, _D + 2), np.float32)
    gat = embed[text_i]                                  # [32, 256, 384]
    egp[:, :, :, :_D] = (gat * amp[:, :, None]).reshape(_B, 2, 128, _D)
    egp[:, :, :, _D] = amp.reshape(_B, 2, 128)
    egp = egp.astype(_BF16)

    tval = np.broadcast_to(
        np.arange(maxspan, dtype=np.float32), (128, maxspan)
    ).copy()

    in_maps = []
    for core in range(_NC):
        ids = order[np.arange(_BPC) * _NC + core]        # batch per slot
        coef_core = stack[ids].copy()                    # [BPC, q, p, c]
        for b in range(_BPC):
            for q in range(2):
                lo0 = spans[b][q][0] * 128
                coef_core[b, q, :, 1] += coef_core[b, q, :, 0] * lo0
        coef_core = (
            coef_core.transpose(2, 0, 1, 3).reshape(128, _BPC * 2 * 2).copy()
        )
        in_maps.append(
            {"coef": coef_core, "tval": tval, "egp": egp[ids].copy()}
        )
    return in_maps, order, td, NTs, spans, maxspan, sched


def kernel(text, durs, embed, total_time):
    global LAST_RESULT
    from concourse.bass_utils import run_bass_kernel_spmd

    Tt = int(total_time)
    embed_f = np.asarray(embed, dtype=np.float32)
    in_maps, order, td, NTs, spans, maxspan, sched = _host_prep(
        text, durs, embed_f, Tt
    )
    nc = _build_program(NTs, spans, maxspan, sched)

    trace = bool(int(os.environ.get("GK_TRACE", "0")))
    res = run_bass_kernel_spmd(
        nc, in_maps, list(range(_NC)), trace=trace
    )
    LAST_RESULT = res

    full = np.empty((_B, Tt, _D), np.float32)
    for core in range(_NC):
        o = res.results[core]["out"]                     # [BPC, NTP, D] bf16
        for b in range(_BPC):
            bid = int(order[b * _NC + core])
            n = min(Tt, NTs[b] * 128)
            full[bid, :n] = o[b, :n].astype(np.float32)
            full[bid, td[bid] :] = embed_f[0]
    return full


if __name__ == "__main__":
    rng = np.random.default_rng(0)
    text = rng.integers(1, _V, size=(_B, _T), dtype=np.int64)
    durs = rng.integers(1, 9, size=(_B, _T), dtype=np.int32)
    embed = rng.normal(size=(_V, _D)).astype(np.float32)
    Tt = int(durs.sum(axis=-1).max())
    o = kernel(text, durs, embed, Tt)
    print("out", o.shape, o.dtype)


# revision 23
# speedup vs baseline: 1.1725x; 1.1725x over previous
"""Gaussian upsampling embedding kernel for Trainium2 (8 NeuronCores).

Data-parallel over the batch dim: 32 batches -> 4 slots per core, with
batches assigned to (core, slot) by sorted total-duration so each slot's
cross-core unions (spans, chunk count) stay tight.

Math (per batch b):
  c_i   = cumsum(durs)_i - durs_i/2          (gaussian centers)
  sig_i = durs_i/2 + 1e-6
  w[t,i] = 1/(sig_i*sqrt(2pi)) * exp(-((t+0.5-c_i)/sig_i)^2/2)
  out[t,:] = sum_i w[t,i]*embed[text_i] / sum_i w[t,i]          (t < total_dur)
  out[t,:] = embed[0]                                           (t >= total_dur)

Device pipeline per slot (engines overlap under Tile):
  ACT : w[i,t] = Derivative_Erf(s_i*tval[t_local] + b'_i) in bf16, over the
        span of t-chunks the char half contributes to (span offset folded
        into b' on host so one short iota serves all spans)
  PE  : O[t,:] = sum over char halves q of w_q[:,tchunk]^T @ Eg_q   (bf16)
        Eg_q[i,:] = amp_i * embed[text_i] + an amp column -> O[:,384] = S
  DVE : recip[t] = 1/S (two 128-row chunks per op via strided PSUM AP)
  DVE/ACT : out = O[:,:384]*recip -> bf16 (psum->sbuf copy fused with
        normalize; work split between the engines by a host-balanced
        schedule — DVE handles whole psum pairs in one tensor_tensor with
        a stride-0 broadcast recip AP, ACT handles single chunks via
        activation Copy+scale); output DMA flushed per psum pair from the
        otherwise-idle GpSimd queue to overlap the store. A dummy
        activation at t=0 preloads the Derivative_Erf table off the
        critical path.

Time-padding rows (t >= total_dur) and the f32 upcast are handled on the
host: those rows are exactly embed[0], so the device never computes them
(rows past the slot's max duration are skipped entirely; rows in computed
chunks may hold NaN from 0 * 1/0 and are overwritten).
"""

import os
import numpy as np
from contextlib import ExitStack

import ml_dtypes

_B, _T, _V, _D = 32, 256, 100, 384
_NC = 8
_BPC = _B // _NC    # batch slots per core
_EPS = np.float32(1e-6)
_MARGIN = 6.0       # |z| beyond which w is dropped (w < 1.6e-8: negligible)
_BF16 = ml_dtypes.bfloat16

# Set by kernel() after each run (for the local test harness).
LAST_RESULT = None


def _build_program(NTs, spans, maxspan, sched):
    """NTs[b] = number of 128-row t-chunks computed for slot b.
    spans[b][q] = (c_lo, c_hi) chunk range half q contributes to (union
    across cores). sched[b][g] in {'dve','act','split'} = normalize
    engine(s) for psum pair g."""
    import concourse.bass as bass
    import concourse.tile as tile
    from concourse import bacc, mybir

    f32 = mybir.dt.float32
    bf16 = mybir.dt.bfloat16
    AF = mybir.ActivationFunctionType
    _af_gauss = (
        AF.Exp if os.environ.get("GK_SIM_AF") else AF.Derivative_Erf
    )

    NTP = max(NTs) * 128

    nc = bacc.Bacc(
        "TRN2",
        target_bir_lowering=False,
        debug=False,
        num_devices=_NC,
    )

    coef = nc.dram_tensor("coef", [128, _BPC * 2 * 2], f32, kind="ExternalInput").ap()
    egp = nc.dram_tensor(
        "egp", [_BPC, 2, 128, _D + 2], bf16, kind="ExternalInput"
    ).ap()
    out = nc.dram_tensor("out", [_BPC, NTP, _D], bf16, kind="ExternalOutput").ap()

    with tile.TileContext(nc) as tc, ExitStack() as ctx:
        const = ctx.enter_context(tc.tile_pool(name="const", bufs=1))
        wpool = ctx.enter_context(tc.tile_pool(name="wT", bufs=8))
        opool = ctx.enter_context(tc.tile_pool(name="osb", bufs=6))
        rpool = ctx.enter_context(tc.tile_pool(name="recip", bufs=10))
        pso = ctx.enter_context(tc.tile_pool(name="pso", bufs=4, space="PSUM"))

        # input DMAs on the Sync queue (ACT-issued DMA wedges the device;
        # Vector can't issue DMAs)
        coef_sb = const.tile([128, _BPC * 2 * 2], f32)
        nc.sync.dma_start(coef_sb[:], coef[:])
        eg_sb = const.tile([128, _BPC * 2 * (_D + 2)], bf16)
        for bb in range(_BPC):
            w0 = bb * 2 * (_D + 2)
            nc.sync.dma_start(
                eg_sb[:, w0 : w0 + 2 * (_D + 2)].rearrange(
                    "p (q d) -> p q d", q=2
                ),
                egp[bb].rearrange("q p d -> p q d"),
            )

        # preload the Derivative_Erf table with a dummy activation so the
        # 1.3us ACT_TABLE_LOAD overlaps the input DMAs (bias from a memset
        # tile: a float bias would pull in a DMA-backed const AP and delay
        # the table load behind the const DMA)
        tiny = const.tile([1, 6], f32)
        nc.vector.memset(tiny[:, 0:4], 0)
        nc.scalar.activation(
            tiny[:, 4:6], tiny[:, 0:2], _af_gauss,
            scale=1.0, bias=tiny[:, 2:3],
        )

        # tval = arange(maxspan) on all partitions (f32 iota exact below
        # 2^24); span starts folded into the bias coefficients on host
        tval_sb = const.tile([128, maxspan], f32)
        nc.gpsimd.iota(
            tval_sb[:], [[1, maxspan]], channel_multiplier=0,
            allow_small_or_imprecise_dtypes=True,
        )

        # warm the PE p-state with junk matmuls during the input DMAs so
        # the real stream starts at full clock instead of ramping
        junk = const.tile([128, 640], bf16)
        nc.vector.memset(junk[:], 0)
        pwarm = pso.tile([128, 512], f32, tag="pso")
        for _ in range(5):
            nc.tensor.matmul(
                pwarm[:], junk[:, :128], junk[:, 128:640],
                start=True, stop=True,
            )

        def cf(b, q, c):
            j = (b * 2 + q) * 2 + c
            return coef_sb[:, j : j + 1]

        def eg(b, q):
            j = (b * 2 + q) * (_D + 2)
            return eg_sb[:, j : j + _D + 2]

        for b in range(_BPC):
            NT = NTs[b]
            # Gaussian eval over each half's contributing span (bf16)
            wT = []
            for q in range(2):
                lo, hi = spans[b][q]
                n = (hi - lo) * 128
                w = wpool.tile([128, n], bf16, tag="wT")
                nc.scalar.activation(
                    w[:], tval_sb[:, :n], _af_gauss,
                    scale=cf(b, q, 0), bias=cf(b, q, 1),
                )
                wT.append(w)

            for g in range((NT + 1) // 2):
                ilist = [i for i in range(2 * g, 2 * g + 2) if i < NT]
                ng = len(ilist)
                po = pso.tile([128, 1024], f32, tag="pso")
                for j, i in enumerate(ilist):
                    dst = po[:, j * 512 : j * 512 + _D + 2]
                    qs = [
                        q
                        for q in range(2)
                        if spans[b][q][0] <= i < spans[b][q][1]
                    ]
                    assert qs, f"t-chunk {i} of slot {b} has no contribution"
                    for k, q in enumerate(qs):
                        o = (i - spans[b][q][0]) * 128
                        nc.tensor.matmul(
                            dst,
                            wT[q][:, o : o + 128],
                            eg(b, q),
                            start=(k == 0),
                            stop=(k == len(qs) - 1),
                        )
                rc = rpool.tile([128, 2], f32, tag="recip")
                nc.vector.reciprocal(
                    rc[:, :ng], po[:, _D : _D + 512 * (ng - 1) + 1 : 512]
                )
                ot = opool.tile([128, ng * _D], bf16, tag="osb")
                m = sched[b][g]            # ACT takes chunks [0, m)
                for j in range(m):
                    nc.scalar.activation(
                        ot[:, j * _D : (j + 1) * _D],
                        po[:, j * 512 : j * 512 + _D],
                        AF.Copy,
                        scale=rc[:, j : j + 1],
                    )
                k = ng - m                 # DVE takes chunks [m, ng)
                if k == 1:
                    nc.vector.tensor_scalar_mul(
                        ot[:, m * _D : (m + 1) * _D],
                        po[:, m * 512 : m * 512 + _D],
                        rc[:, m : m + 1],
                    )
                elif k >= 2:
                    nc.vector.tensor_tensor(
                        ot[:, m * _D :].rearrange("p (j d) -> p j d", d=_D),
                        po[:, m * 512 : m * 512 + k * 512].rearrange(
                            "p (j d) -> p j d", j=k
                        )[:, :, 0:_D],
                        rc[:, m : m + k].unsqueeze(2).broadcast_to(
                            [128, k, _D]
                        ),
                        mybir.AluOpType.mult,
                    )
                nc.gpsimd.dma_start(
                    out[b, 2 * g * 128 : (2 * g + ng) * 128].rearrange(
                        "(i p) d -> p i d", p=128
                    ),
                    ot[:].rearrange("p (i d) -> p i d", d=_D),
                )

    nc.compile()
    return nc


def _host_prep(text, durs, embed, Tt):
    """Sorted slot assignment, per-core input maps, spans, schedule."""
    text_i = np.asarray(text).astype(np.int64)          # [32, 256]
    durs_f = np.asarray(durs).astype(np.float32)        # [32, 256]
    embed = np.asarray(embed, dtype=np.float32)         # [100, 384]

    td = np.asarray(durs).astype(np.int64).sum(axis=-1)  # [32]
    order = np.argsort(td, kind="stable")                # slot-major ranks
    # batch at (core c, slot b) = order[b*8 + c]
    NTs = []
    for b in range(_BPC):
        mx = int(td[order[b * _NC : (b + 1) * _NC]].max())
        NTs.append(-(-mx // 128))

    csum = np.cumsum(durs_f, axis=-1, dtype=np.float32)
    c = csum - durs_f / 2.0                             # centers
    sig = durs_f / 2.0 + _EPS
    sq2 = np.float32(np.sqrt(2.0))
    s_coef = (1.0 / (sig * sq2)).astype(np.float32)
    b_coef = ((0.5 - c) / (sig * sq2)).astype(np.float32)
    amp = (1.0 / (2.0 * sq2 * sig)).astype(np.float32)

    # contribution spans per (slot, char-half) on the 128-chunk grid,
    # unioned across the 8 cores (SPMD-shared program)
    lo_t = (c - _MARGIN * sig).reshape(_B, 2, 128).min(axis=2)
    hi_t = (c + _MARGIN * sig + 1).reshape(_B, 2, 128).max(axis=2)
    spans = []
    for b in range(_BPC):
        ids = order[b * _NC : (b + 1) * _NC]
        NT = NTs[b]
        row = []
        for q in range(2):
            lo = max(0.0, float(lo_t[ids, q].min()))
            hi = min(float(NT * 128), float(hi_t[ids, q].max()))
            c_lo = max(0, min(int(lo) // 128, NT - 1))
            c_hi = max(c_lo + 1, min(-(-int(hi) // 128), NT))
            row.append((c_lo, c_hi))
        # coverage check: every chunk must get at least one matmul
        for i in range(NT):
            assert any(r[0] <= i < r[1] for r in row), (b, i, row)
        spans.append(tuple(row))
    spans = tuple(spans)
    maxspan = max((hi - lo) * 128 for row in spans for (lo, hi) in row)

    # normalize engine schedule per psum pair: sched[b][g] = m = number of
    # chunks ACT takes (from the front); DVE fuses the rest in one
    # tensor_tensor. Greedy balance of estimated busy ns (measured: ACT
    # chunk ~755, DVE pair-fused ~950, single ~613, recip ~165). ACT also
    # evaluates the gaussians.
    act_t = sum(
        (hi - lo) * 128 * 0.833 + 400.0 for row in spans for (lo, hi) in row
    )
    dve_t = 0.0
    sched = []
    for b in range(_BPC):
        row = []
        for g in range(-(-NTs[b] // 2)):
            ng = min(2, NTs[b] - 2 * g)
            best, cost = None, None
            for m in range(ng + 1):
                k = ng - m
                a = m * 755.0
                v = 165.0 + (0.0 if k == 0 else (613.0 if k == 1 else 950.0))
                c = max(act_t + a, dve_t + v)
                if cost is None or c < cost:
                    best, cost, ba, bv = m, c, a, v
            row.append(best)
            act_t += ba
            dve_t += bv
        sched.append(row)

    # coef layout: [128 partitions, (b, q, c)] with c = (s, b'),
    # b' = b + s * span_start so the short local iota can be used
    stack = np.stack([s_coef, b_coef], axis=-1)          # [32, 256, 2]
    stack = stack.reshape(_B, 2, 128, 2)                 # [32, q, p, c]

    # gathered, amplitude-folded embeddings + amp column (row-sum), bf16
    egp = np.zeros((_B, 2, 128, _D + 2), np.float32)
    gat = embed[text_i]                                  # [32, 256, 384]
    egp[:, :, :, :_D] = (gat * amp[:, :, None]).reshape(_B, 2, 128, _D)
    egp[:, :, :, _D] = amp.reshape(_B, 2, 128)
    egp = egp.astype(_BF16)

    in_maps = []
    for core in range(_NC):
        ids = order[np.arange(_BPC) * _NC + core]        # batch per slot
        coef_core = stack[ids].copy()                    # [BPC, q, p, c]
        for b in range(_BPC):
            for q in range(2):
                lo0 = spans[b][q][0] * 128
                coef_core[b, q, :, 1] += coef_core[b, q, :, 0] * lo0
        coef_core = (
            coef_core.transpose(2, 0, 1, 3).reshape(128, _BPC * 2 * 2).copy()
        )
        in_maps.append(
            {"coef": coef_core, "egp": egp[ids].copy()}
        )
    return in_maps, order, td, NTs, spans, maxspan, sched


def kernel(text, durs, embed, total_time):
    global LAST_RESULT
    from concourse.bass_utils import run_bass_kernel_spmd

    Tt = int(total_time)
    embed_f = np.asarray(embed, dtype=np.float32)
    in_maps, order, td, NTs, spans, maxspan, sched = _host_prep(
        text, durs, embed_f, Tt
    )
    nc = _build_program(NTs, spans, maxspan, sched)

    trace = bool(int(os.environ.get("GK_TRACE", "0")))
    res = run_bass_kernel_spmd(
        nc, in_maps, list(range(_NC)), trace=trace
    )
    LAST_RESULT = res

    full = np.empty((_B, Tt, _D), np.float32)
    for core in range(_NC):
        o = res.results[core]["out"]                     # [BPC, NTP, D] bf16
        for b in range(_BPC):
            bid = int(order[b * _NC + core])
            n = min(Tt, NTs[b] * 128)
            full[bid, :n] = o[b, :n].astype(np.float32)
            full[bid, td[bid] :] = embed_f[0]
    return full


if __name__ == "__main__":
    rng = np.random.default_rng(0)
    text = rng.integers(1, _V, size=(_B, _T), dtype=np.int64)
    durs = rng.integers(1, 9, size=(_B, _T), dtype=np.int32)
    embed = rng.normal(size=(_V, _D)).astype(np.float32)
    Tt = int(durs.sum(axis=-1).max())
    o = kernel(text, durs, embed, Tt)
    print("out", o.shape, o.dtype)


# revision 26
# speedup vs baseline: 1.2140x; 1.0354x over previous
"""Gaussian upsampling embedding kernel for Trainium2 (8 NeuronCores).

Data-parallel over the batch dim: 32 batches -> 4 slots per core, with
batches assigned to (core, slot) by sorted total-duration so each slot's
cross-core unions (spans, chunk count) stay tight.

Math (per batch b):
  c_i   = cumsum(durs)_i - durs_i/2          (gaussian centers)
  sig_i = durs_i/2 + 1e-6
  w[t,i] = 1/(sig_i*sqrt(2pi)) * exp(-((t+0.5-c_i)/sig_i)^2/2)
  out[t,:] = sum_i w[t,i]*embed[text_i] / sum_i w[t,i]          (t < total_dur)
  out[t,:] = embed[0]                                           (t >= total_dur)

Device pipeline per slot (engines overlap under Tile):
  ACT : w[i,t] = Derivative_Erf(s_i*tval[t_local] + b'_i) in bf16, over the
        span of t-chunks the char half contributes to (span offset folded
        into b' on host so one short iota serves all spans)
  PE  : O[t,:] = sum over char halves q of w_q[:,tchunk]^T @ Eg_q   (bf16)
        Eg_q[i,:] = amp_i * embed[text_i] + an amp column -> O[:,384] = S
  DVE : recip[t] = 1/S (two 128-row chunks per op via strided PSUM AP)
  DVE/ACT : out = O[:,:384]*recip -> bf16 (psum->sbuf copy fused with
        normalize; work split between the engines by a host-balanced
        schedule — DVE handles whole psum pairs in one tensor_tensor with
        a stride-0 broadcast recip AP, ACT handles single chunks via
        activation Copy+scale); output DMA flushed per psum pair from the
        otherwise-idle GpSimd queue to overlap the store. A dummy
        activation at t=0 preloads the Derivative_Erf table off the
        critical path.

Time-padding rows (t >= total_dur) and the f32 upcast are handled on the
host: those rows are exactly embed[0], so the device never computes them
(rows past the slot's max duration are skipped entirely; rows in computed
chunks may hold NaN from 0 * 1/0 and are overwritten).
"""

import os
import numpy as np
from contextlib import ExitStack

import ml_dtypes

_B, _T, _V, _D = 32, 256, 100, 384
_NC = 8
_BPC = _B // _NC    # batch slots per core
_EPS = np.float32(1e-6)
_MARGIN = 6.0       # |z| beyond which w is dropped (w < 1.6e-8: negligible)
_BF16 = ml_dtypes.bfloat16

# Set by kernel() after each run (for the local test harness).
LAST_RESULT = None


def _build_program(NTs, spans, maxspan, sched):
    """NTs[b] = number of 128-row t-chunks computed for slot b.
    spans[b][q] = (c_lo, c_hi) chunk range half q contributes to (union
    across cores). sched[b][g] in {'dve','act','split'} = normalize
    engine(s) for psum pair g."""
    import concourse.bass as bass
    import concourse.tile as tile
    from concourse import bacc, mybir

    f32 = mybir.dt.float32
    bf16 = mybir.dt.bfloat16
    AF = mybir.ActivationFunctionType
    _af_gauss = (
        AF.Exp if os.environ.get("GK_SIM_AF") else AF.Derivative_Erf
    )

    NTP = max(NTs) * 128

    nc = bacc.Bacc(
        "TRN2",
        target_bir_lowering=False,
        debug=False,
        num_devices=_NC,
    )

    coef = nc.dram_tensor("coef", [128, _BPC * 2 * 2], f32, kind="ExternalInput").ap()
    egp = nc.dram_tensor(
        "egp", [_BPC, 2, 128, _D + 2], bf16, kind="ExternalInput"
    ).ap()
    out = nc.dram_tensor("out", [_BPC, NTP, _D], bf16, kind="ExternalOutput").ap()

    with tile.TileContext(nc) as tc, ExitStack() as ctx:
        const = ctx.enter_context(tc.tile_pool(name="const", bufs=1))
        wpool = ctx.enter_context(tc.tile_pool(name="wT", bufs=8))
        opool = ctx.enter_context(tc.tile_pool(name="osb", bufs=6))
        rpool = ctx.enter_context(tc.tile_pool(name="recip", bufs=10))
        pso = ctx.enter_context(tc.tile_pool(name="pso", bufs=4, space="PSUM"))

        # input DMAs on the Sync queue (ACT-issued DMA wedges the device;
        # Vector can't issue DMAs)
        coef_sb = const.tile([128, _BPC * 2 * 2], f32)
        nc.sync.dma_start(coef_sb[:], coef[:])
        eg_sb = const.tile([128, _BPC * 2 * (_D + 2)], bf16)
        for bb in range(_BPC):
            w0 = bb * 2 * (_D + 2)
            nc.sync.dma_start(
                eg_sb[:, w0 : w0 + 2 * (_D + 2)].rearrange(
                    "p (q d) -> p q d", q=2
                ),
                egp[bb].rearrange("q p d -> p q d"),
            )

        # preload the Derivative_Erf table with a dummy activation so the
        # 1.3us ACT_TABLE_LOAD overlaps the input DMAs (bias from a memset
        # tile: a float bias would pull in a DMA-backed const AP and delay
        # the table load behind the const DMA)
        tiny = const.tile([1, 6], f32)
        nc.vector.memset(tiny[:, 0:4], 0)
        nc.scalar.activation(
            tiny[:, 4:6], tiny[:, 0:2], _af_gauss,
            scale=1.0, bias=tiny[:, 2:3],
        )

        # tval = arange(maxspan) on all partitions (f32 iota exact below
        # 2^24); span starts folded into the bias coefficients on host
        tval_sb = const.tile([128, maxspan], f32)
        nc.gpsimd.iota(
            tval_sb[:], [[1, maxspan]], channel_multiplier=0,
            allow_small_or_imprecise_dtypes=True,
        )

        def cf(b, q, c):
            j = (b * 2 + q) * 2 + c
            return coef_sb[:, j : j + 1]

        def eg(b, q):
            j = (b * 2 + q) * (_D + 2)
            return eg_sb[:, j : j + _D + 2]

        # all gaussian evals hoisted up front: ACT head-of-line blocking
        # otherwise stalls the next slot's matmuls behind this slot's
        # normalize copies
        wTs = []
        for b in range(_BPC):
            wT = []
            for q in range(2):
                lo, hi = spans[b][q]
                n = (hi - lo) * 128
                w = wpool.tile([128, n], bf16, tag="wT")
                nc.scalar.activation(
                    w[:], tval_sb[:, :n], _af_gauss,
                    scale=cf(b, q, 0), bias=cf(b, q, 1),
                )
                wT.append(w)
            wTs.append(wT)

        nflush = 0
        for b in range(_BPC):
            NT = NTs[b]
            wT = wTs[b]
            for g in range((NT + 1) // 2):
                ilist = [i for i in range(2 * g, 2 * g + 2) if i < NT]
                ng = len(ilist)
                po = pso.tile([128, 1024], f32, tag="pso")
                for j, i in enumerate(ilist):
                    dst = po[:, j * 512 : j * 512 + _D + 2]
                    qs = [
                        q
                        for q in range(2)
                        if spans[b][q][0] <= i < spans[b][q][1]
                    ]
                    assert qs, f"t-chunk {i} of slot {b} has no contribution"
                    for k, q in enumerate(qs):
                        o = (i - spans[b][q][0]) * 128
                        nc.tensor.matmul(
                            dst,
                            wT[q][:, o : o + 128],
                            eg(b, q),
                            start=(k == 0),
                            stop=(k == len(qs) - 1),
                        )
                rc = rpool.tile([128, 2], f32, tag="recip")
                nc.vector.reciprocal(
                    rc[:, :ng], po[:, _D : _D + 512 * (ng - 1) + 1 : 512]
                )
                ot = opool.tile([128, ng * _D], bf16, tag="osb")
                m = sched[b][g]            # ACT takes chunks [0, m)
                for j in range(m):
                    nc.scalar.activation(
                        ot[:, j * _D : (j + 1) * _D],
                        po[:, j * 512 : j * 512 + _D],
                        AF.Copy,
                        scale=rc[:, j : j + 1],
                    )
                k = ng - m                 # DVE takes chunks [m, ng)
                if k == 1:
                    nc.vector.tensor_scalar_mul(
                        ot[:, m * _D : (m + 1) * _D],
                        po[:, m * 512 : m * 512 + _D],
                        rc[:, m : m + 1],
                    )
                elif k >= 2:
                    nc.vector.tensor_tensor(
                        ot[:, m * _D :].rearrange("p (j d) -> p j d", d=_D),
                        po[:, m * 512 : m * 512 + k * 512].rearrange(
                            "p (j d) -> p j d", j=k
                        )[:, :, 0:_D],
                        rc[:, m : m + k].unsqueeze(2).broadcast_to(
                            [128, k, _D]
                        ),
                        mybir.AluOpType.mult,
                    )
                feng = nc.gpsimd if nflush % 2 == 0 else nc.sync
                nflush += 1
                feng.dma_start(
                    out[b, 2 * g * 128 : (2 * g + ng) * 128].rearrange(
                        "(i p) d -> p i d", p=128
                    ),
                    ot[:].rearrange("p (i d) -> p i d", d=_D),
                )

    nc.compile()
    return nc


def _host_prep(text, durs, embed, Tt):
    """Sorted slot assignment, per-core input maps, spans, schedule."""
    text_i = np.asarray(text).astype(np.int64)          # [32, 256]
    durs_f = np.asarray(durs).astype(np.float32)        # [32, 256]
    embed = np.asarray(embed, dtype=np.float32)         # [100, 384]

    td = np.asarray(durs).astype(np.int64).sum(axis=-1)  # [32]
    order = np.argsort(td, kind="stable")                # slot-major ranks
    # batch at (core c, slot b) = order[b*8 + c]
    NTs = []
    for b in range(_BPC):
        mx = int(td[order[b * _NC : (b + 1) * _NC]].max())
        NTs.append(-(-mx // 128))

    csum = np.cumsum(durs_f, axis=-1, dtype=np.float32)
    c = csum - durs_f / 2.0                             # centers
    sig = durs_f / 2.0 + _EPS
    sq2 = np.float32(np.sqrt(2.0))
    s_coef = (1.0 / (sig * sq2)).astype(np.float32)
    b_coef = ((0.5 - c) / (sig * sq2)).astype(np.float32)
    amp = (1.0 / (2.0 * sq2 * sig)).astype(np.float32)

    # contribution spans per (slot, char-half) on the 128-chunk grid,
    # unioned across the 8 cores (SPMD-shared program)
    lo_t = (c - _MARGIN * sig).reshape(_B, 2, 128).min(axis=2)
    hi_t = (c + _MARGIN * sig + 1).reshape(_B, 2, 128).max(axis=2)
    spans = []
    for b in range(_BPC):
        ids = order[b * _NC : (b + 1) * _NC]
        NT = NTs[b]
        row = []
        for q in range(2):
            lo = max(0.0, float(lo_t[ids, q].min()))
            hi = min(float(NT * 128), float(hi_t[ids, q].max()))
            c_lo = max(0, min(int(lo) // 128, NT - 1))
            c_hi = max(c_lo + 1, min(-(-int(hi) // 128), NT))
            row.append((c_lo, c_hi))
        # coverage check: every chunk must get at least one matmul
        for i in range(NT):
            assert any(r[0] <= i < r[1] for r in row), (b, i, row)
        spans.append(tuple(row))
    spans = tuple(spans)
    maxspan = max((hi - lo) * 128 for row in spans for (lo, hi) in row)

    # normalize engine schedule per psum pair: sched[b][g] = m = number of
    # chunks ACT takes (from the front); DVE fuses the rest in one
    # tensor_tensor. A small discrete-event model of the pipeline picks
    # the assignment that minimizes each pair's completion time (measured
    # costs: ACT chunk ~755, DVE pair-fused ~950, single ~613, recip ~165,
    # matmul ~390; ACT starts after the 8 hoisted gaussian evals).
    act_free = 8000.0 + sum(
        (hi - lo) * 128 * 0.833 + 400.0 for row in spans for (lo, hi) in row
    )
    dve_free = 0.0
    mm_t = 9300.0
    sched = []
    for b in range(_BPC):
        row = []
        for g in range(-(-NTs[b] // 2)):
            ilist = [i for i in range(2 * g, 2 * g + 2) if i < NTs[b]]
            ng = len(ilist)
            nmm = sum(
                1
                for i in ilist
                for q in range(2)
                if spans[b][q][0] <= i < spans[b][q][1]
            )
            mm_t += nmm * 390.0
            rec = max(dve_free, mm_t) + 165.0
            best, cost = None, None
            for m in range(ng + 1):
                k = ng - m
                a_done = (max(act_free, rec) + m * 755.0) if m else act_free
                if k == 0:
                    d_done, fin = rec, a_done
                elif k == 1:
                    d_done = rec + 613.0
                    fin = max(a_done, d_done)
                else:
                    d_done = rec + 950.0
                    fin = d_done
                c = (max(fin, 0), max(a_done, d_done))
                if cost is None or c < cost:
                    best, cost, ba, bd = m, c, a_done, d_done
            row.append(best)
            act_free, dve_free = ba, bd
        sched.append(row)

    # coef layout: [128 partitions, (b, q, c)] with c = (s, b'),
    # b' = b + s * span_start so the short local iota can be used
    stack = np.stack([s_coef, b_coef], axis=-1)          # [32, 256, 2]
    stack = stack.reshape(_B, 2, 128, 2)                 # [32, q, p, c]

    # gathered, amplitude-folded embeddings + amp column (row-sum), bf16
    egp = np.zeros((_B, 2, 128, _D + 2), np.float32)
    gat = embed[text_i]                                  # [32, 256, 384]
    egp[:, :, :, :_D] = (gat * amp[:, :, None]).reshape(_B, 2, 128, _D)
    egp[:, :, :, _D] = amp.reshape(_B, 2, 128)
    egp = egp.astype(_BF16)

    in_maps = []
    for core in range(_NC):
        ids = order[np.arange(_BPC) * _NC + core]        # batch per slot
        coef_core = stack[ids].copy()                    # [BPC, q, p, c]
        for b in range(_BPC):
            for q in range(2):
                lo0 = spans[b][q][0] * 128
                coef_core[b, q, :, 1] += coef_core[b, q, :, 0] * lo0
        coef_core = (
            coef_core.transpose(2, 0, 1, 3).reshape(128, _BPC * 2 * 2).copy()
        )
        in_maps.append(
            {"coef": coef_core, "egp": egp[ids].copy()}
        )
    return in_maps, order, td, NTs, spans, maxspan, sched


def kernel(text, durs, embed, total_time):
    global LAST_RESULT
    from concourse.bass_utils import run_bass_kernel_spmd

    Tt = int(total_time)
    embed_f = np.asarray(embed, dtype=np.float32)
    in_maps, order, td, NTs, spans, maxspan, sched = _host_prep(
        text, durs, embed_f, Tt
    )
    nc = _build_program(NTs, spans, maxspan, sched)

    trace = bool(int(os.environ.get("GK_TRACE", "0")))
    res = run_bass_kernel_spmd(
        nc, in_maps, list(range(_NC)), trace=trace
    )
    LAST_RESULT = res

    full = np.empty((_B, Tt, _D), np.float32)
    for core in range(_NC):
        o = res.results[core]["out"]                     # [BPC, NTP, D] bf16
        for b in range(_BPC):
            bid = int(order[b * _NC + core])
            n = min(Tt, NTs[b] * 128)
            full[bid, :n] = o[b, :n].astype(np.float32)
            full[bid, td[bid] :] = embed_f[0]
    return full


if __name__ == "__main__":
    rng = np.random.default_rng(0)
    text = rng.integers(1, _V, size=(_B, _T), dtype=np.int64)
    durs = rng.integers(1, 9, size=(_B, _T), dtype=np.int32)
    embed = rng.normal(size=(_V, _D)).astype(np.float32)
    Tt = int(durs.sum(axis=-1).max())
    o = kernel(text, durs, embed, Tt)
    print("out", o.shape, o.dtype)
